# revision 20
# baseline (speedup 1.0000x reference)
"""Trainium2 Bass kernel for nn_DGNRNetwork (2-layer TransformerConv GNN).

v2 strategy (8 NeuronCores, SPMD single NEFF):
  - Node tables in AllGather-row order g(n) = owner(n)*nag + local_pos(n);
    local_pos == position in the per-core (padA,padB)-sorted node order, so
    conv outputs are written back with plain contiguous DMA (no scatter).
  - k and v projections are interleaved into ONE fp16 table row
    [k(h,c) 128 | v(c,h) 128] = 512B, so each edge needs a single 512B
    gather descriptor (full DMA-bus rate, half the descriptor count).
  - k-bias is dropped entirely (softmax is shift-invariant per dst);
    v-bias is folded in after normalization (sum of attn weights is 1).
    Padding slots gather a dedicated all-zero dummy row -> logits 0,
    corrected by subtracting npad*exp(-SHIFT) from the softmax denominator,
    eliminating all mask tensors and mask multiplies.
  - Edge-phase elementwise runs fully in fp16 with packed innermost axes
    (DVE 2x mode); segment reductions are tree-halving tensor_tensor adds
    (TensorReduce has no fast mode, TensorTensor does).
  - All matmuls run in fp16 (4x PE rate); PSUM->SBUF copies ride the
    Activation engine.
  - conv1->conv2 crosses cores with a 4-chunk AllGather of the dm-masked
    h1 shard, overlapped with the conv2 node phase.
"""

import os
import sys

import numpy as np

for _p in ("/opt/trn_rl_repo", "/root/.axon_site/_ro/trn_rl_repo"):
    if os.path.isdir(_p) and _p not in sys.path:
        sys.path.append(_p)

# problem constants
N = 50000
E = 800000
B = 1000
IN_DIM = 64
HID = 32
HEADS = 4
D1 = HID * HEADS  # 128
OUT_DIM = 5
NCORES = 8
P = 128

PAD_LIST = [0, 1, 2, 3, 4, 5, 6, 7, 8, 9, 10, 11, 12, 13, 14, 15, 16,
            18, 20, 22, 24, 28, 32, 40, 48, 64]
CHUNK_SLOTS = int(os.environ.get("K_CHUNK_SLOTS", "24"))
GATHER_COLS = 8    # slot columns per dma_gather call (1024 indices)
NODE_CHUNK = 512
SHIFT = 2.0        # exp(alpha*invs - SHIFT); cancels in the softmax ratio
AG_CHUNKS = 4

# feature permutation: v columns stored channel-major (c,h) so the
# attn-weight broadcast lands on a packed innermost axis.
PERM_CH = np.arange(D1).reshape(HEADS, HID).T.reshape(-1)  # f'=c*4+h <- f=h*32+c


def _round_up(a, m):
    return (a + m - 1) // m * m


# --------------------------------------------------------------------------
# host-side layout
# --------------------------------------------------------------------------

def build_layout(edge_index, global_indices, n_nodes, n_b, n_cores=NCORES):
    src = np.asarray(edge_index[0], dtype=np.int64)
    dst = np.asarray(edge_index[1], dtype=np.int64)
    gi = np.asarray(global_indices, dtype=np.int64)

    deg = np.bincount(dst, minlength=n_nodes).astype(np.int64)

    # ownership: deal active nodes round-robin in degree order (edge balance)
    active = np.nonzero(deg > 0)[0]
    order_by_deg = active[np.argsort(deg[active], kind="stable")]
    owner = np.full(n_nodes, -1, dtype=np.int32)
    for c in range(n_cores):
        owner[order_by_deg[c::n_cores]] = c
    inactive = np.nonzero(deg == 0)[0]
    for c in range(n_cores):
        owner[inactive[c::n_cores]] = c

    n_act = np.array([int(((owner == c) & (deg > 0)).sum()) for c in range(n_cores)])
    n_inact = np.array([int(((owner == c) & (deg == 0)).sum()) for c in range(n_cores)])
    ND = int((n_act.max() + P - 1) // P)
    nag = _round_up(ND * P + int(n_inact.max()) + 1, P)
    NAG = n_cores * nag
    HS = 4 * nag
    assert HS < 32768 and NAG - HS < 32768, nag
    assert NAG % NODE_CHUNK == 0
    DUMA = nag - 1            # dummy zero row, A half (global row nag-1)
    DUMB_G = HS + nag - 1     # dummy zero row, B half (global row)

    # need pads before sorting: split edges at HS by *owner stripe* of src:
    # g(src) < HS  <=>  owner(src) < 4
    srcA = owner[src] < 4
    degA = np.bincount(dst[srcA], minlength=n_nodes).astype(np.int64)
    degB = deg - degA

    pad_arr = np.array(PAD_LIST)

    def pad_of(arr):
        return pad_arr[np.searchsorted(pad_arr, arr, side="left")]

    assert int(degA.max()) <= PAD_LIST[-1] and int(degB.max()) <= PAD_LIST[-1]
    kA_all = np.zeros(n_nodes, dtype=np.int64)
    kB_all = np.zeros(n_nodes, dtype=np.int64)
    kA_all[active] = pad_of(degA[active])
    kB_all[active] = pad_of(degB[active])

    # per-core node order -> local_pos (identity with nd layout)
    core_sorted = []
    local_pos = np.full(n_nodes, -1, dtype=np.int64)
    for c in range(n_cores):
        mine = active[owner[active] == c]
        o = np.lexsort((mine, -kB_all[mine], -kA_all[mine]))
        s = mine[o]
        core_sorted.append(s)
        local_pos[s] = np.arange(len(s))
        ia = inactive[owner[inactive] == c]
        local_pos[ia] = ND * P + np.arange(len(ia))

    g_all = owner.astype(np.int64) * nag + local_pos

    # edges sorted by (dst, g(src)); A half first per dst
    gsrc = g_all[src]
    order = np.lexsort((gsrc, dst))
    sgsrc = gsrc[order].astype(np.int64)
    rowptr = np.zeros(n_nodes + 1, dtype=np.int64)
    np.cumsum(deg, out=rowptr[1:])

    # unified per-column pads (max across cores)
    CA = np.zeros(ND, dtype=np.int64)
    CB = np.zeros(ND, dtype=np.int64)
    for c in range(n_cores):
        s = core_sorted[c]
        for w in range(ND):
            seg = s[w * P : (w + 1) * P]
            if len(seg):
                CA[w] = max(CA[w], int(kA_all[seg].max()))
                CB[w] = max(CB[w], int(kB_all[seg].max()))
    colbaseA = np.zeros(ND + 1, dtype=np.int64)
    np.cumsum(CA, out=colbaseA[1:])
    KA = int(colbaseA[-1])
    colbaseB = np.zeros(ND + 1, dtype=np.int64)
    np.cumsum(CB, out=colbaseB[1:])
    KB = int(colbaseB[-1])

    def build_runs(CW, other):
        runs = []
        w = 0
        while w < ND:
            D = int(CW[w])
            w2 = w
            while (
                w2 < ND and int(CW[w2]) == D
                and ((other[w2] > 0) == (other[w] > 0))
            ):
                w2 += 1
            if D > 0:
                runs.append((w, w2 - w, D, bool(other[w] > 0)))
            w = w2
        return runs

    runsA = [(w0, cw, D, False) for (w0, cw, D, _) in build_runs(CA, np.zeros(ND))]
    runsB = build_runs(CB, CA)

    # split runs at AllGather group boundaries so each window group can
    # finalize (and start its collective) as soon as its windows finish
    WGL = sorted(set([max(1, (35 * ND) // 100), max(2, (67 * ND) // 100)]))
    def split_runs(runs):
        out = []
        for (w0, cw, D, sec) in runs:
            cuts = [b for b in WGL if w0 < b < w0 + cw]
            lo = w0
            for b in cuts:
                out.append((lo, b - lo, D, sec))
                lo = b
            out.append((lo, w0 + cw - lo, D, sec))
        return out
    runsA = split_runs(runsA)
    runsB = split_runs(runsB)

    gi_owner = owner[gi]
    bc = max(int(np.bincount(gi_owner, minlength=n_cores).max()), 1)
    BC = _round_up(bc, P)

    def wrap16(flat):
        a = np.zeros((16, len(flat) // 16), np.int16)
        i = np.arange(len(flat))
        a[i % 16, i // 16] = flat.astype(np.int16)
        return np.tile(a, (8, 1))

    esh = float(np.exp(-SHIFT))
    cores = []
    for c in range(n_cores):
        idxA = np.full(KA * P, DUMA, dtype=np.int64)
        idxB = np.full(KB * P, DUMB_G - HS, dtype=np.int64)
        q_idx = np.zeros((P, ND), dtype=np.int32)
        npad = np.tile(
            (CA[None, :] + CB[None, :]).astype(np.float64), (P, 1)) * esh
        dm_sel = np.full((P, ND), -1, dtype=np.int64)  # node id per nd slot
        nodes = core_sorted[c]
        n = len(nodes)
        j = np.arange(n)
        p = j % P
        w = j // P
        q_idx[p, w] = g_all[nodes]
        dm_sel[p, w] = nodes
        npad[p, w] = ((CA[w] - degA[nodes]) + (CB[w] - degB[nodes])) * esh
        for idxf, dg, off, sub, colbase in (
            (idxA, degA, None, 0, colbaseA),
            (idxB, degB, degA, HS, colbaseB),
        ):
            d = dg[nodes]
            tot = int(d.sum())
            if tot == 0:
                continue
            pe = np.repeat(p, d)
            within = np.arange(tot) - np.repeat(np.cumsum(d) - d, d)
            ce = np.repeat(colbase[w], d) + within
            base = rowptr[nodes] if off is None else rowptr[nodes] + off[nodes]
            e0 = np.repeat(base, d) + within
            vals = sgsrc[e0] - sub
            idxf[ce * P + pe] = vals

        sel = np.nonzero(gi_owner == c)[0]
        nsel = len(sel)
        x12_idx = np.zeros((P, BC // P), dtype=np.int32)
        xl_idx = np.full((P, BC // P), ND * P, dtype=np.int32)
        osc = np.full((P, BC // P), n_b, dtype=np.int32)
        jj = np.arange(nsel)
        gn = gi[sel]
        x12_idx[jj % P, jj // P] = g_all[gn].astype(np.int32)
        xl_idx[jj % P, jj // P] = local_pos[gn].astype(np.int32)
        osc[jj % P, jj // P] = sel.astype(np.int32)

        cores.append(
            dict(idxA16=wrap16(idxA), idxB16=wrap16(idxB),
                 q_idx=q_idx, npad=np.repeat(npad, HEADS, axis=1).astype(np.float32),
                 dm_sel=dm_sel,
                 x12_idx=x12_idx, xl_idx=xl_idx, oscat_idx=osc, sel=sel)
        )

    meta = dict(
        WGL=WGL,
        runsA=runsA, runsB=runsB, CA=CA, CB=CB,
        colbaseA=colbaseA, colbaseB=colbaseB, KA=KA, KB=KB, ND=ND,
        nag=nag, NAG=NAG, HS=HS, BC=BC, DUMA=DUMA, DUMB_G=DUMB_G,
        owner=owner, local_pos=local_pos, g_all=g_all,
        n_cores=n_cores, n_nodes=n_nodes, n_b=n_b,
    )
    return meta, cores


# --------------------------------------------------------------------------
# bass program
# --------------------------------------------------------------------------

def build_bass(meta):
    import concourse.bass as bass
    import concourse.tile as tile
    from concourse import bacc, mybir
    from concourse.masks import make_identity

    f32 = mybir.dt.float32
    f16 = mybir.dt.float16
    i32 = mybir.dt.int32
    i16 = mybir.dt.int16
    OP = mybir.AluOpType
    ACT = mybir.ActivationFunctionType

    n_b = meta["n_b"]
    ND, KA, KB = meta["ND"], meta["KA"], meta["KB"]
    nag, NAG, HS, BC = meta["nag"], meta["NAG"], meta["HS"], meta["BC"]
    runsA, runsB = meta["runsA"], meta["runsB"]
    colbaseA, colbaseB = meta["colbaseA"], meta["colbaseB"]
    invs = float(1.0 / np.sqrt(np.float32(HID)))

    # AllGather / conv2 chunking over local rows: three window groups
    WGL = meta["WGL"]
    bounds = [0] + WGL + [ND]
    wgs = [(bounds[i], bounds[i + 1]) for i in range(len(bounds) - 1)]
    ag_rows = [(bounds[i] * P, bounds[i + 1] * P) for i in range(len(bounds) - 1)]
    ag_rows[-1] = (ag_rows[-1][0], nag)

    nc = bacc.Bacc(None, target_bir_lowering=False,
                   dynamic_dma_scratch_size=65536)

    def ein(name, shape, dtype=f32):
        return nc.dram_tensor(name, shape, dtype, kind="ExternalInput")

    xg = ein("xg", [IN_DIM + 1, NAG], f16)
    w1p = ein("w1p", [IN_DIM + 1, HID], f16)
    w2p = ein("w2p", [HID, HID], f16)
    b2c = ein("b2c", [HID, 1])
    wq1p = ein("wq1p", [HID, D1], f16)
    wkv1p = ein("wkv1p", [HID, 2 * D1], f16)
    wq2p = ein("wq2p", [D1, D1], f16)
    wkv2p = ein("wkv2p", [D1, 2 * D1], f16)
    bq1r = ein("bq1r", [P, D1], f16)
    bq2r = ein("bq2r", [P, D1], f16)
    bv1r = ein("bv1r", [P, D1], f16)
    bv2r = ein("bv2r", [P, D1], f16)
    ow1 = ein("ow1", [HID, OUT_DIM])
    ow2 = ein("ow2", [D1, OUT_DIM])
    ow3 = ein("ow3", [D1, OUT_DIM])
    obr = ein("obr", [P, OUT_DIM])
    idxA_d = ein("idxA16", [P, KA * 8], i16)
    idxB_d = ein("idxB16", [P, KB * 8], i16) if KB else None
    qidx_d = ein("q_idx", [P, ND], i32)
    npad_d = ein("npad", [P, ND * HEADS])
    dmnd_d = ein("dm_ndr", [P, ND * D1], f16)
    x12_d = ein("x12_idx", [P, BC // P], i32)
    xl_d = ein("xl_idx", [P, BC // P], i32)
    osc_d = ein("oscat_idx", [P, BC // P], i32)

    outp = nc.dram_tensor("outp", [n_b + P, OUT_DIM], f32, kind="ExternalOutput")

    ht = nc.dram_tensor("ht", [NAG, HID], f16)
    kv1t = nc.dram_tensor("kv1t", [NAG, 2 * D1], f16)
    kv2t = nc.dram_tensor("kv2t", [NAG, 2 * D1], f16)
    h1shard = nc.dram_tensor("h1shard", [nag, D1], f16)
    h1m = nc.dram_tensor("h1m", [nag, D1], f16)
    h2shard = nc.dram_tensor("h2shard", [nag, D1], f16)
    h1ag = [nc.dram_tensor(f"h1ag{i}", [8 * (r1 - r0), D1], f16,
                           addr_space="Shared")
            for i, (r0, r1) in enumerate(ag_rows)]

    with tile.TileContext(nc) as tc:
        with (
            tc.tile_pool(name="const", bufs=1) as cpool,
            tc.tile_pool(name="work", bufs=3) as wpool,
            tc.tile_pool(name="slot", bufs=int(os.environ.get("K_SLOT_BUFS", "5"))) as spool,
            tc.tile_pool(name="big", bufs=1) as bpool,
            tc.tile_pool(name="tmp", bufs=2) as tpool,
            tc.tile_pool(name="reg", bufs=1) as rpool,
            tc.tile_pool(name="ps", bufs=int(os.environ.get("K_PS_BUFS", "2")), space="PSUM") as pspool,
            tc.tile_pool(name="pst", bufs=int(os.environ.get("K_PST_BUFS", "2")), space="PSUM") as pstpool,
        ):
            def load_const(dram, shape, dtype=f32):
                t = cpool.tile(shape, dtype, tag=f"c_{dram.name}")
                nc.sync.dma_start(out=t[:], in_=dram[:, :])
                return t

            w1s = load_const(w1p, [IN_DIM + 1, HID], f16)
            w2s = load_const(w2p, [HID, HID], f16)
            b2s = load_const(b2c, [HID, 1])
            wq1s = load_const(wq1p, [HID, D1], f16)
            wkv1s = load_const(wkv1p, [HID, 2 * D1], f16)
            wq2s = load_const(wq2p, [D1, D1], f16)
            wkv2s = load_const(wkv2p, [D1, 2 * D1], f16)
            bq1s = load_const(bq1r, [P, D1], f16)
            bq2s = load_const(bq2r, [P, D1], f16)
            bv1s = load_const(bv1r, [P, D1], f16)
            bv2s = load_const(bv2r, [P, D1], f16)
            ow1s = load_const(ow1, [HID, OUT_DIM])
            ow2s = load_const(ow2, [D1, OUT_DIM])
            ow3s = load_const(ow3, [D1, OUT_DIM])
            obs = load_const(obr, [P, OUT_DIM])
            idxAs = load_const(idxA_d, [P, KA * 8], i16)
            idxBs = load_const(idxB_d, [P, KB * 8], i16) if KB else None
            qidxs = load_const(qidx_d, [P, ND], i32)
            npads = load_const(npad_d, [P, ND * HEADS])
            x12s = load_const(x12_d, [P, BC // P], i32)
            xls = load_const(xl_d, [P, BC // P], i32)
            oscs = load_const(osc_d, [P, BC // P], i32)

            ident = cpool.tile([P, P], f16)
            make_identity(nc, ident[:])
            zrow = cpool.tile([P, 2 * D1], f16)
            nc.vector.memset(zrow[:], 0.0)
            shiftc = cpool.tile([P, 1], f32)
            nc.vector.memset(shiftc[:], -SHIFT)

            Areg = rpool.tile([P, ND * D1], f16)   # q1 then q2
            Breg = rpool.tile([P, ND * D1], f16)   # conv out
            dreg = rpool.tile([P, ND * HEADS], f32)

            # zero shard tails (inactive-node rows + dummy rows)
            tail = nag - ND * P
            for shard0 in (h1shard, h1m, h2shard):
                nc.gpsimd.dma_start(
                    out=shard0[ND * P : nag, :].rearrange(
                        "(j p) f -> p j f", p=P),
                    in_=zrow[:, : tail // P * D1].rearrange(
                        "p (j f) -> p j f", f=D1),
                )

            # ---------------- conv1 node phase (g-ordered) ----------------
            ksb2 = [None]
            hsb2p = [None]
            for c0 in range(0, NAG, NODE_CHUNK):
                half = (c0 // NODE_CHUNK) % 2
                xt_t = wpool.tile([IN_DIM + 1, NODE_CHUNK], f16, tag="xt")
                nc.sync.dma_start(out=xt_t[:], in_=xg[:, c0 : c0 + NODE_CHUNK])
                ps1 = pspool.tile([HID, NODE_CHUNK], f32, tag="mm32")
                nc.tensor.matmul(ps1[:], w1s[:], xt_t[:], start=True, stop=True)
                h1e = wpool.tile([HID, NODE_CHUNK], f16, tag="h1e")
                nc.scalar.activation(h1e[:], ps1[:], ACT.Relu)
                ps2 = pspool.tile([HID, NODE_CHUNK], f32, tag="mm32")
                nc.tensor.matmul(ps2[:], w2s[:], h1e[:], start=True, stop=True)
                hTt = wpool.tile([HID, NODE_CHUNK], f16, tag="hT")
                nc.scalar.activation(hTt[:], ps2[:], ACT.Relu, bias=b2s[:, 0:1])

                pkv = pspool.tile([P, NODE_CHUNK // P * 2 * D1], f32, tag="mmk")
                for j in range(NODE_CHUNK // P):
                    nc.tensor.matmul(
                        pkv[:, j * 2 * D1 : (j + 1) * 2 * D1],
                        hTt[:, j * P : (j + 1) * P],
                        wkv1s[:],
                        start=True, stop=True,
                    )
                if half == 0:
                    ksb_new = wpool.tile(
                        [P, 2 * NODE_CHUNK // P * 2 * D1], f16, tag="ksb")
                    ksb2[0] = ksb_new
                ksb = ksb2[0]
                ko = half * (NODE_CHUNK // P * 2 * D1)
                nc.vector.tensor_copy(
                    ksb[:, ko : ko + NODE_CHUNK // P * 2 * D1], pkv[:])
                if half == 1:
                    nc.sync.dma_start(
                        out=kv1t[c0 - NODE_CHUNK : c0 + NODE_CHUNK, :]
                        .rearrange("(j p) f -> p j f", p=P),
                        in_=ksb[:].rearrange("p (j f) -> p j f", f=2 * D1),
                    )

                phb = pstpool.tile([P, NODE_CHUNK // P * HID], f16, tag="t")
                for j in range(NODE_CHUNK // P):
                    nc.tensor.transpose(
                        phb[:, j * HID : (j + 1) * HID],
                        hTt[:, j * P : (j + 1) * P],
                        ident[0:HID, 0:HID],
                    )
                if half == 0:
                    hsb_new = wpool.tile(
                        [P, 2 * NODE_CHUNK // P * HID], f16, tag="hsb")
                    hsb2p[0] = hsb_new
                hsb = hsb2p[0]
                ho = half * (NODE_CHUNK // P * HID)
                nc.vector.tensor_copy(
                    hsb[:, ho : ho + NODE_CHUNK // P * HID], phb[:])
                if half == 1:
                    nc.sync.dma_start(
                        out=ht[c0 - NODE_CHUNK : c0 + NODE_CHUNK, :]
                        .rearrange("(j p) f -> p j f", p=P),
                        in_=hsb[:].rearrange("p (j f) -> p j f", f=HID),
                    )

            # zero the dummy gather rows of kv1t
            for r in (meta["DUMA"], meta["DUMB_G"]):
                nc.gpsimd.dma_start(out=kv1t[r : r + 1, :], in_=zrow[0:1, :])

            # ---------------- q1 (nd order) ----------------
            hq = cpool.tile([P, ND * HID], f16)
            for w in range(ND):
                nc.gpsimd.indirect_dma_start(
                    out=hq[:, w * HID : (w + 1) * HID],
                    out_offset=None,
                    in_=ht[:, :],
                    in_offset=bass.IndirectOffsetOnAxis(
                        ap=qidxs[:, w : w + 1], axis=0),
                )
            QB = 4

            def q_project(src_reg, src_w, wq_s, bq_s):
                """Areg[:, w*D1:(w+1)*D1] = src^T @ wq + bq for all nd cols."""
                for w0 in range(0, ND, QB):
                    qn = min(QB, ND - w0)
                    pt = pstpool.tile([src_w, QB * P], f16, tag="t")
                    for j in range(qn):
                        nc.tensor.transpose(
                            pt[:, j * P : (j + 1) * P],
                            src_reg[:, (w0 + j) * src_w : (w0 + j + 1) * src_w],
                            ident[:],
                        )
                    hqT = tpool.tile([src_w, QB * P], f16, tag="hqT")
                    nc.scalar.copy(hqT[:, : qn * P], pt[:, : qn * P])
                    pq = pstpool.tile([P, QB * D1], f32, tag="t")
                    for j in range(qn):
                        nc.tensor.matmul(
                            pq[:, j * D1 : (j + 1) * D1],
                            hqT[:, j * P : (j + 1) * P], wq_s[:],
                            start=True, stop=True,
                        )
                    nc.scalar.copy(
                        Areg[:, w0 * D1 : (w0 + qn) * D1], pq[:, : qn * D1])
                a3 = Areg[:].rearrange("p (n f) -> p n f", f=D1)
                nc.vector.tensor_tensor(
                    a3, a3,
                    bq_s[:].unsqueeze(1).to_broadcast([P, ND, D1]), OP.add)

            q_project(hq, HID, wq1s, bq1s)

            # ---------------- edge phase ----------------
            def gather_cols(dst_tile, cc_total, table, idx_tile, gcol0):
                for p0 in range(0, cc_total, GATHER_COLS):
                    pc = min(GATHER_COLS, cc_total - p0)
                    ni = pc * P
                    i0 = (gcol0 + p0) * P
                    nc.gpsimd.dma_gather(
                        out_ap=dst_tile[:, p0 * 2 * D1 : (p0 + pc) * 2 * D1]
                        .rearrange("p (s f) -> p s f", f=2 * D1),
                        in_ap=table,
                        idxs_ap=idx_tile[:, i0 // 16 : (i0 + ni) // 16],
                        num_idxs=ni,
                        num_idxs_reg=ni,
                        elem_size=2 * D1,
                    )

            def tree_halve(view4, D, axis_len_fn, add):
                """view4(lo, hi) -> AP pair slices along t; performs in-place
                halving sum into t=0."""
                cur = D
                while cur > 1:
                    if cur % 2 == 1:
                        add(view4(0, 1), view4(cur - 1, cur))
                        cur -= 1
                    h = cur // 2
                    add(view4(0, h), view4(h, cur))
                    cur = h

            def edge_phase(kvtab, group_hook):
                for gi_, (gw0, gw1) in enumerate(wgs):
                    passes = [
                        (runsA, colbaseA, idxAs, kvtab[0:HS, :]),
                        (runsB, colbaseB, idxBs, kvtab[HS:NAG, :]),
                    ]
                    for runs, colbase, idx_t, tab in passes:
                      for rw0, rcw, D, second in runs:
                        if rw0 < gw0 or rw0 >= gw1:
                            continue
                        cw_max = max(1, CHUNK_SLOTS // D)
                        for w0 in range(rw0, rw0 + rcw, cw_max):
                            cw = min(cw_max, rw0 + rcw - w0)
                            cc = cw * D
                            gc0 = int(colbase[w0])
                            nd0 = w0
                            big = D > CHUNK_SLOTS
                            pool_s = bpool if big else spool
                            stag = "bigslot" if big else "slot"
                            kvt = pool_s.tile([P, cc * 2 * D1], f16, tag=stag)
                            gather_cols(kvt, cc, tab, idx_t, gc0)
                            g4 = kvt[:].rearrange(
                                "p (w t g) -> p w t g", t=D, g=2 * D1)
                            k4 = g4[:, :, :, 0:D1]
                            qb = (
                                Areg[:, nd0 * D1 : (nd0 + cw) * D1]
                                .rearrange("p (w f) -> p w f", f=D1)
                                .unsqueeze(2)
                                .to_broadcast([P, cw, D, D1])
                            )
                            nc.vector.tensor_tensor(k4, k4, qb, OP.mult)
                            # head-sum tree over c (32 per head)
                            s4 = kvt[:].rearrange(
                                "p (s g) -> p s g", g=2 * D1)[:, :, 0:D1]
                            s5 = s4.rearrange("p s (h c) -> p s h c", c=HID)
                            ch = HID
                            while ch > 1:
                                h2 = ch // 2
                                nc.vector.tensor_tensor(
                                    s5[:, :, :, 0:h2], s5[:, :, :, 0:h2],
                                    s5[:, :, :, h2:ch], OP.add)
                                ch = h2
                            al = spool.tile([P, cc * HEADS], f16, tag="alpha")
                            nc.scalar.activation(
                                al[:].rearrange("p (s h) -> p s h", h=HEADS)
                                .unsqueeze(3),
                                s5[:, :, :, 0:1],
                                ACT.Exp, scale=invs, bias=shiftc[:, 0:1])
                            # attn * v  (v stored channel-major (c,h))
                            v4 = g4[:, :, :, D1 : 2 * D1]
                            vs = kvt[:].rearrange(
                                "p (s g) -> p s g", g=2 * D1)[:, :, D1 : 2 * D1]
                            v5 = vs.rearrange("p s (c h) -> p s c h", h=HEADS)
                            eb = (
                                al[:].rearrange("p (s h) -> p s h", h=HEADS)
                                .unsqueeze(2)
                                .to_broadcast([P, cc, HID, HEADS])
                            )
                            nc.vector.tensor_tensor(v5, v5, eb, OP.mult)
                            # window-sum tree over t for v
                            tree_halve(
                                lambda lo, hi: v4[:, :, lo:hi, :], D, None,
                                lambda a, b: nc.vector.tensor_tensor(
                                    a, a, b, OP.add))
                            bs = (Breg[:, nd0 * D1 : (nd0 + cw) * D1]
                                  .rearrange("p (w f) -> p w f", f=D1)
                                  .unsqueeze(2))
                            if second:
                                nc.vector.tensor_tensor(
                                    bs, bs, v4[:, :, 0:1, :], OP.add)
                            else:
                                nc.vector.tensor_copy(bs, v4[:, :, 0:1, :])
                            # window-sum tree over t for denom
                            a4 = al[:].rearrange(
                                "p (w t h) -> p w t h", t=D, h=HEADS)
                            tree_halve(
                                lambda lo, hi: a4[:, :, lo:hi, :], D, None,
                                lambda a, b: nc.vector.tensor_tensor(
                                    a, a, b, OP.add))
                            ds = (dreg[:, nd0 * HEADS : (nd0 + cw) * HEADS]
                                  .rearrange("p (w h) -> p w h", h=HEADS)
                                  .unsqueeze(2))
                            if second:
                                nc.vector.tensor_tensor(
                                    ds, ds, a4[:, :, 0:1, :], OP.add)
                            else:
                                nc.vector.tensor_copy(ds, a4[:, :, 0:1, :])
                    group_hook(gi_)

            def finalize_group(bv_s, gw0, gw1):
                nw = gw1 - gw0
                dsl = dreg[:, gw0 * HEADS : gw1 * HEADS]
                nc.vector.tensor_tensor(
                    dsl, dsl, npads[:, gw0 * HEADS : gw1 * HEADS], OP.subtract)
                nc.vector.tensor_scalar_add(dsl, dsl, 1e-16)
                nc.vector.reciprocal(dsl, dsl)
                bsl = Breg[:, gw0 * D1 : gw1 * D1]
                b4 = bsl.rearrange("p (n c h) -> p n c h", h=HEADS, c=HID)
                rb = (
                    dsl.rearrange("p (n h) -> p n h", h=HEADS)
                    .unsqueeze(2)
                    .to_broadcast([P, nw, HID, HEADS])
                )
                nc.vector.tensor_tensor(b4, b4, rb, OP.mult)
                b3 = bsl.rearrange("p (n f) -> p n f", f=D1)
                nc.vector.tensor_tensor(
                    b3, b3, bv_s[:].unsqueeze(1).to_broadcast([P, nw, D1]),
                    OP.add)
                nc.scalar.activation(bsl, bsl, ACT.Relu)

            # ---------------- conv1 ----------------
            def conv1_hook(g):
                gw0, gw1 = wgs[g]
                r0, r1 = ag_rows[g]
                finalize_group(bv1s, gw0, gw1)
                nc.sync.dma_start(
                    out=h1shard[gw0 * P : gw1 * P, :].rearrange(
                        "(w p) f -> p w f", p=P),
                    in_=Breg[:, gw0 * D1 : gw1 * D1].rearrange(
                        "p (w f) -> p w f", f=D1),
                )
                dmg = cpool.tile([P, (gw1 - gw0) * D1], f16, tag="dmg")
                nc.sync.dma_start(
                    out=dmg[:], in_=dmnd_d[:, gw0 * D1 : gw1 * D1])
                bsl = Breg[:, gw0 * D1 : gw1 * D1]
                nc.vector.tensor_tensor(bsl, bsl, dmg[:], OP.mult)
                nc.sync.dma_start(
                    out=h1m[gw0 * P : gw1 * P, :].rearrange(
                        "(w p) f -> p w f", p=P),
                    in_=bsl.rearrange("p (w f) -> p w f", f=D1),
                )
                nc.gpsimd.collective_compute(
                    "AllGather",
                    mybir.AluOpType.bypass,
                    replica_groups=[list(range(meta["n_cores"]))],
                    ins=[h1m[r0:r1, :]],
                    outs=[h1ag[g][:, :]],
                )

            edge_phase(kv1t, conv1_hook)

            # ---------------- conv2 node phase (chunk-ordered) ----------------
            for i, (r0, r1) in enumerate(ag_rows):
                ln = r1 - r0
                for c in range(meta["n_cores"]):
                    for off in range(0, ln, NODE_CHUNK):
                        L = min(NODE_CHUNK, ln - off)
                        nj = L // P
                        hsb2 = wpool.tile([P, NODE_CHUNK // P * D1], f16,
                                          tag="hsb2")
                        nc.sync.dma_start(
                            out=hsb2[:, : nj * D1].rearrange(
                                "p (j f) -> p j f", f=D1),
                            in_=h1ag[i][c * ln + off : c * ln + off + L, :]
                            .rearrange("(j p) f -> p j f", p=P),
                        )
                        ptr = pstpool.tile([P, NODE_CHUNK], f16, tag="t")
                        for j in range(nj):
                            nc.tensor.transpose(
                                ptr[:, j * D1 : (j + 1) * D1],
                                hsb2[:, j * D1 : (j + 1) * D1], ident[:])
                        h1T = tpool.tile([P, NODE_CHUNK], f16, tag="h1T")
                        nc.vector.tensor_copy(h1T[:, : nj * D1], ptr[:, : nj * D1])
                        pkv = pspool.tile([P, NODE_CHUNK // P * 2 * D1], f32,
                                          tag="mmk")
                        for j in range(nj):
                            nc.tensor.matmul(
                                pkv[:, j * 2 * D1 : (j + 1) * 2 * D1],
                                h1T[:, j * P : (j + 1) * P], wkv2s[:],
                                start=True, stop=True,
                            )
                        ksb = wpool.tile([P, NODE_CHUNK // P * 2 * D1], f16,
                                         tag="ksb")
                        nc.vector.tensor_copy(ksb[:, : nj * 2 * D1],
                                              pkv[:, : nj * 2 * D1])
                        g0 = c * nag + r0 + off
                        nc.sync.dma_start(
                            out=kv2t[g0 : g0 + L, :].rearrange(
                                "(j p) f -> p j f", p=P),
                            in_=ksb[:, : nj * 2 * D1].rearrange(
                                "p (j f) -> p j f", f=2 * D1),
                        )

            # ---------------- q2 from masked Breg ----------------
            q_project(Breg, D1, wq2s, bq2s)

            # ---------------- conv2 ----------------
            def conv2_hook(g):
                gw0, gw1 = wgs[g]
                finalize_group(bv2s, gw0, gw1)
                nc.sync.dma_start(
                    out=h2shard[gw0 * P : gw1 * P, :].rearrange(
                        "(w p) f -> p w f", p=P),
                    in_=Breg[:, gw0 * D1 : gw1 * D1].rearrange(
                        "p (w f) -> p w f", f=D1),
                )

            edge_phase(kv2t, conv2_hook)

            # ---------------- head ----------------
            x1g = cpool.tile([P, BC // P * HID], f16)
            x2g = cpool.tile([P, BC // P * D1], f16)
            x3g = cpool.tile([P, BC // P * D1], f16)
            for j in range(BC // P):
                nc.gpsimd.indirect_dma_start(
                    out=x1g[:, j * HID : (j + 1) * HID],
                    out_offset=None, in_=ht[:, :],
                    in_offset=bass.IndirectOffsetOnAxis(
                        ap=x12s[:, j : j + 1], axis=0),
                )
                nc.gpsimd.indirect_dma_start(
                    out=x2g[:, j * D1 : (j + 1) * D1],
                    out_offset=None, in_=h1shard[:, :],
                    in_offset=bass.IndirectOffsetOnAxis(
                        ap=xls[:, j : j + 1], axis=0),
                )
                nc.gpsimd.indirect_dma_start(
                    out=x3g[:, j * D1 : (j + 1) * D1],
                    out_offset=None, in_=h2shard[:, :],
                    in_offset=bass.IndirectOffsetOnAxis(
                        ap=xls[:, j : j + 1], axis=0),
                )
            for j in range(BC // P):
                p1 = pstpool.tile([HID, P], f16, tag="t")
                nc.tensor.transpose(
                    p1[:], x1g[:, j * HID : (j + 1) * HID], ident[:])
                x1T = wpool.tile([HID, P], f32, tag="x1T")
                nc.scalar.copy(x1T[:], p1[:])
                p2 = pstpool.tile([P, P], f16, tag="t")
                nc.tensor.transpose(
                    p2[:], x2g[:, j * D1 : (j + 1) * D1], ident[:])
                x2T = wpool.tile([P, P], f32, tag="x2T")
                nc.scalar.copy(x2T[:], p2[:])
                p3 = pstpool.tile([P, P], f16, tag="t")
                nc.tensor.transpose(
                    p3[:], x3g[:, j * D1 : (j + 1) * D1], ident[:])
                x3T = wpool.tile([P, P], f32, tag="x3T")
                nc.scalar.copy(x3T[:], p3[:])
                po = pstpool.tile([P, OUT_DIM], f32, tag="t")
                nc.tensor.matmul(po[:], x1T[:], ow1s[:], start=True, stop=False)
                nc.tensor.matmul(po[:], x2T[:], ow2s[:], start=False, stop=False)
                nc.tensor.matmul(po[:], x3T[:], ow3s[:], start=False, stop=True)
                osb = wpool.tile([P, OUT_DIM], f32, tag="osb")
                nc.vector.scalar_tensor_tensor(
                    osb[:], po[:], 0.0, obs[:], OP.bypass, OP.add)
                nc.gpsimd.indirect_dma_start(
                    out=outp[:, :],
                    out_offset=bass.IndirectOffsetOnAxis(
                        ap=oscs[:, j : j + 1], axis=0),
                    in_=osb[:],
                    in_offset=None,
                )

    nc.finalize()
    return nc


# --------------------------------------------------------------------------
# host packing
# --------------------------------------------------------------------------

def pack_inputs(inputs, meta, cores):
    f32 = np.float32
    f16 = np.float16
    NAG = meta["NAG"]
    g_all = meta["g_all"]
    ND = meta["ND"]

    x = np.asarray(inputs["x"], dtype=f32)
    dm = np.asarray(inputs["dm_mask"], dtype=f32).reshape(-1)

    xg = np.zeros((IN_DIM + 1, NAG), f16)
    xg[:IN_DIM, g_all] = x.T.astype(f16)
    xg[IN_DIM, :] = 1.0

    wk1 = np.asarray(inputs["c1_wk"], f32)
    wv1 = np.asarray(inputs["c1_wv"], f32)[:, PERM_CH]
    wk2 = np.asarray(inputs["c2_wk"], f32)[PERM_CH, :]
    wv2 = np.asarray(inputs["c2_wv"], f32)[PERM_CH][:, PERM_CH]
    ow = np.asarray(inputs["out_w"], f32)

    common = {
        "xg": xg,
        "w1p": np.vstack([np.asarray(inputs["enc_w1"], f32),
                          np.asarray(inputs["enc_b1"], f32)[None, :]]).astype(f16),
        "w2p": np.asarray(inputs["enc_w2"], f16),
        "b2c": np.asarray(inputs["enc_b2"], f32)[:, None],
        "wq1p": np.asarray(inputs["c1_wq"], f16),
        "wkv1p": np.hstack([wk1, wv1]).astype(f16),
        "wq2p": np.asarray(inputs["c2_wq"], f32)[PERM_CH, :].astype(f16),
        "wkv2p": np.hstack([wk2, wv2]).astype(f16),
        "bq1r": np.tile(np.asarray(inputs["c1_bq"], f16)[None, :], (P, 1)),
        "bq2r": np.tile(np.asarray(inputs["c2_bq"], f16)[None, :], (P, 1)),
        "bv1r": np.tile(np.asarray(inputs["c1_bv"], f32)[PERM_CH][None, :],
                        (P, 1)).astype(f16),
        "bv2r": np.tile(np.asarray(inputs["c2_bv"], f32)[PERM_CH][None, :],
                        (P, 1)).astype(f16),
        "ow1": ow[:HID],
        "ow2": ow[HID : HID + D1][PERM_CH, :],
        "ow3": ow[HID + D1 :][PERM_CH, :],
        "obr": np.tile(np.asarray(inputs["out_b"], f32)[None, :], (P, 1)),
    }

    in_maps = []
    for c, L in enumerate(cores):
        valid = L["dm_sel"] >= 0
        dm_nd = np.where(valid, dm[np.where(valid, L["dm_sel"], 0)], 0.0)
        m = dict(common)
        m.update(
            idxA16=L["idxA16"], idxB16=L["idxB16"],
            q_idx=L["q_idx"], npad=L["npad"],
            dm_ndr=np.repeat(dm_nd.astype(f16), D1, axis=1),
            x12_idx=L["x12_idx"], xl_idx=L["xl_idx"],
            oscat_idx=L["oscat_idx"],
        )
        if meta["KB"] == 0:
            m.pop("idxB16")
        in_maps.append({k: np.ascontiguousarray(v) for k, v in m.items()})
    return in_maps


_CACHE = {}


def kernel(**inputs):
    from concourse.bass_utils import run_bass_kernel_spmd

    meta, cores = build_layout(
        inputs["edge_index"], inputs["global_indices"], N, B
    )
    nc = build_bass(meta)
    in_maps = pack_inputs(inputs, meta, cores)

    trace = bool(int(os.environ.get("KERNEL_TRACE", "0")))
    res = run_bass_kernel_spmd(
        nc, in_maps, core_ids=list(range(NCORES)), trace=trace,
    )
    if trace and res.exec_time_ns is not None:
        print(f"HW exec time: {res.exec_time_ns} ns")
        _CACHE["exec_time_ns"] = res.exec_time_ns
        _CACHE["res"] = res

    out = np.zeros((B, OUT_DIM), np.float32)
    for c, L in enumerate(cores):
        sel = L["sel"]
        out[sel] = res.results[c]["outp"][sel]
    return out


if __name__ == "__main__":
    import jax

    cpu = jax.devices("cpu")[0]
    sys.path.insert(0, "/root/problem")
    import reference

    with jax.default_device(cpu):
        inputs = {k: np.asarray(v) for k, v in reference.setup_inputs().items()}
        expected = np.asarray(reference.reference(**inputs))
    got = kernel(**inputs)
    err = np.abs(got - expected).max() / (np.abs(expected).max() + 1e-12)
    print("rel err:", err)
